# revision 15
# baseline (speedup 1.0000x reference)
"""AttentionWithRotary on 8 Trainium2 cores.

Math: reference applies raw (no-softmax) attention:
    out = ((rope(xWq^T+bq) @ rope(xWk^T+bk)^T)/sqrt(hd)) @ (xWv^T+bv) @ Wo^T + bo
Since there is no softmax, associativity gives per head:
    out_h = Q_r @ (K_r^T @ V) / sqrt(hd)
which turns the S x S score matrix into a hd x hd one.

Sharding: data-parallel on batch (2) x tensor-parallel on heads (4 heads/core).
Each core computes a row-parallel partial of the output projection; partials
are summed on the host (the "all-reduce" of row-parallel TP) and bo added.

Performance structure (see cost model: HWDGE issue is ~625ns per DMA
instruction and PE re-throttles to half clock after any idle gap):
  - every operand is packed on the host so each SBUF load is one (or a few
    0.5MB-chunk) large 2D DMAs with long contiguous rows;
  - block 0 is computed chunk-major (K first, 4 concurrent PSUM accumulation
    banks, consuming 0.5MB weight/x chunks as they arrive) so the PE never
    waits for a full 2MB operand during the initial load;
  - a short burst of warm-up matmuls on a memset tile keeps the PE busy (and
    at full clock) while the first chunks stream in;
  - output is written bf16, one [128,2048] row per DMA (partials summed on
    host in f32), with the last row split in 512-wide tiles to cut the tail.

Per-core layouts (prepped on host, all bf16):
  xP   [512,8192]   block blk rows: xP[blk*128+p, k*512+j] = x[b][blk*512+j, k*128+p]
  wqP/wkP/wvP [128,8192]  wP[p, k*512+m] = w.T[k*128+p, m]; q/k rows pair-split
                 permuted per head (64 even pair elems then 64 odd); q scaled
                 by 1/sqrt(hd)
  woP  [128,8192]  woP[p, h*2048+c] = wo_slice.T[h*128+p, c]
  c2   [128,2048]  [cosT; cosT]        (cosT = freqs_cos^T, [64,2048])
  s2x  [128,2048]  [sinT; -sinT]
  trigP [2048,1024] row s: [cn4(s) | sn4x(s)] (per-head [cos,cos]/[sin,-sin])
  bqc [128,4]; bkc/bv_rep [128,512] broadcast bias rows (k bias permuted).

On-device per core:
  QT (rope) [512,2048] transposed layout; K_r,V natural [2048,512] streamed;
  Mh[h] = K_r^T V accumulated in one PSUM bank across all seq tiles;
  N_h = Mh[h]^T wo_h; out rows = sum_h QT[h]^T @ N_h, written bf16.
"""
import numpy as np
import ml_dtypes
from contextlib import ExitStack

import concourse.bacc as bacc
import concourse.tile as tile
import concourse.mybir as mybir
from concourse.bass_utils import run_bass_kernel_spmd

BF16 = mybir.dt.bfloat16
F32 = mybir.dt.float32
NPBF = ml_dtypes.bfloat16

S = 2048
DIN = 2048
NH = 16
HD = 128
B = 2
NCORES = 8
TP = 4                 # head-parallel ways
NHL = NH // TP         # 4 heads per core
DLOC = NHL * HD        # 512 local head dims
BLK = 512
NBLK = S // BLK        # 4 seq blocks
KT = DIN // 128        # 16 contraction tiles
P = 128
NCH = 4                # 0.5MB load chunks (4 k-tiles each)
KCH = KT // NCH

_NC_CACHE = None


def _emit(nc, tc, ctx, d, out_d, dbg=None, reps=1, loop_n=0):
    wpool = ctx.enter_context(tc.tile_pool(name="w", bufs=1))
    xpool = ctx.enter_context(tc.tile_pool(name="x", bufs=2))
    tpool = ctx.enter_context(tc.tile_pool(name="trig", bufs=5))
    qpool = ctx.enter_context(tc.tile_pool(name="qtr", bufs=1))
    kvpool = ctx.enter_context(tc.tile_pool(name="kv", bufs=4))
    tmp = ctx.enter_context(tc.tile_pool(name="tmp", bufs=3))
    osb = ctx.enter_context(tc.tile_pool(name="osb", bufs=3))
    psum = ctx.enter_context(tc.tile_pool(name="ps", bufs=7, space="PSUM"))
    mps = ctx.enter_context(tc.tile_pool(name="mps", bufs=1, space="PSUM"))

    def load_xt(blk, chunks=1):
        # one wide DMA per seq block (rows of xP are fully contiguous);
        # chunked for block 0 so the PE ramps while the rest streams
        t = xpool.tile([P, KT * BLK], BF16, name="xt", tag="xt")
        n = KT * BLK // chunks
        for j in range(chunks):
            nc.gpsimd.dma_start(
                t[:, j * n:(j + 1) * n],
                d["xP"][blk * P:(blk + 1) * P, j * n:(j + 1) * n])
        return t

    # --- PE warm-up: keep the tensor engine busy (and the clock ramping)
    # while the first weight/x chunks stream in; spins rotate the (as yet
    # unused) mh PSUM bank so the main pool's bank rotation is undisturbed ---
    spin = wpool.tile([P, BLK], BF16, name="spin", tag="spin")
    nc.vector.memset(spin[:], 0.125)
    for _ in range(16):
        sp_ps = mps.tile([P, DLOC], F32, name="sp_ps", tag="mh")
        nc.tensor.matmul(sp_ps[:], spin[:, 0:P], spin[:],
                         start=True, stop=True, skip_group_check=True)

    # --- resident constants; SP-queue issue order == arrival order, laid
    # out just-in-time against block 0's chunk-major compute order (K, Q
    # h0-2, Q h3, V) and the DVE rope order (K ropes, then Q ropes) ---
    ck = 2048  # 0.5MB chunk = 4 k-tiles
    wk_all = wpool.tile([P, KT * DLOC], BF16, name="wk_all", tag="wk_all")
    for j in range(NCH):
        nc.sync.dma_start(wk_all[:, j * ck:(j + 1) * ck],
                          d["wkP"][:, j * ck:(j + 1) * ck])
    xt0 = load_xt(0, chunks=NCH)
    wq_all = wpool.tile([P, KT * DLOC], BF16, name="wq_all", tag="wq_all")
    for j in range(NCH):
        nc.sync.dma_start(wq_all[:, j * ck:(j + 1) * ck],
                          d["wqP"][:, j * ck:(j + 1) * ck])
    bkc = wpool.tile([P, DLOC], BF16, name="bkc", tag="bkc")
    nc.sync.dma_start(bkc[:], d["bkc"][:])
    trig0 = []
    for m in range(4):
        trig0.append(tpool.tile([P, 2 * DLOC], BF16, name="trig", tag="trig"))
    for m in range(2):
        nc.sync.dma_start(trig0[m][:], d["trigP"][m * P:(m + 1) * P, :])
    wv_all = wpool.tile([P, KT * DLOC], BF16, name="wv_all", tag="wv_all")
    for j in range(2):
        nc.sync.dma_start(wv_all[:, j * 4096:(j + 1) * 4096],
                          d["wvP"][:, j * 4096:(j + 1) * 4096])
    for m in range(2, 4):
        nc.sync.dma_start(trig0[m][:], d["trigP"][m * P:(m + 1) * P, :])
    c2 = wpool.tile([P, S], BF16, name="c2", tag="c2")
    nc.sync.dma_start(c2[:], d["c2"][:])
    s2x = wpool.tile([P, S], BF16, name="s2x", tag="s2x")
    nc.sync.dma_start(s2x[:], d["s2x"][:])
    bqc = wpool.tile([P, NHL], BF16, name="bqc", tag="bqc")
    nc.sync.dma_start(bqc[:], d["bqc"][:])
    bv_rep = wpool.tile([P, DLOC], BF16, name="bv_rep", tag="bv_rep")
    nc.sync.dma_start(bv_rep[:], d["bv_rep"][:])
    bias = {"bv_rep": bv_rep, "bkc": bkc}
    # xt1 preloaded on the SP queue so its 2MB doesn't displace block-0's
    # weights on the bus (Pool would issue it immediately); needed ~45us in
    xt1 = xpool.tile([P, KT * BLK], BF16, name="xt", tag="xt")
    nc.sync.dma_start(xt1[:], d["xP"][P:2 * P, :])
    wo_all = wpool.tile([P, NHL * DIN], BF16, name="wo_all", tag="wo_all")
    nc.sync.dma_start(wo_all[:], d["woP"][:])

    qtr = []
    for h in range(NHL):
        t = qpool.tile([P, S], BF16, name=f"qtr{h}", tag=f"qtr{h}")
        qtr.append(t)

    def pair_view(t):
        return t.rearrange("p (h u j) -> p h u j", h=NHL, u=2, j=64)

    if loop_n:
        with tc.For_i(0, loop_n, 1) as _i:
            _emit_compute(nc, tc, d, out_d, dbg, tpool, kvpool, tmp,
                          osb, psum, mps, bias, bqc, c2, s2x, wq_all,
                          wk_all, wv_all, wo_all, qtr, pair_view, load_xt,
                          None, None)
    else:
        for _rep in range(reps):
            first = _rep == 0
            _emit_compute(nc, tc, d, out_d, dbg, tpool, kvpool, tmp,
                          osb, psum, mps, bias, bqc, c2, s2x, wq_all,
                          wk_all, wv_all, wo_all, qtr, pair_view, load_xt,
                          {0: xt0, 1: xt1} if first else None,
                          trig0 if first else None)


def _emit_compute(nc, tc, d, out_d, dbg, tpool, kvpool, tmp, osb, psum,
                  mps, bias, bqc, c2, s2x, wq_all, wk_all, wv_all,
                  wo_all, qtr, pair_view, load_xt, xts, trig0):
    ADD = mybir.AluOpType.add
    MULT = mybir.AluOpType.mult
    mh_ps = mps.tile([P, DLOC], F32, name="mh_ps", tag="mh")

    def emit_mh(kr, vt, mg):
        if mg == 0:
            # start=True clears the whole PSUM bank, so only the first
            # matmul may carry it; the critical section pins the order
            # of the four first-writes within the shared bank.
            with tc.tile_critical():
                for h in range(NHL):
                    h0, h1 = h * 128, (h + 1) * 128
                    nc.tensor.matmul(mh_ps[:, h0:h1], vt[:, h0:h1],
                                     kr[:, h0:h1], start=(h == 0),
                                     stop=False, skip_group_check=True)
        else:
            for h in range(NHL):
                h0, h1 = h * 128, (h + 1) * 128
                nc.tensor.matmul(mh_ps[:, h0:h1], vt[:, h0:h1],
                                 kr[:, h0:h1], start=False,
                                 stop=(mg == 4 * NBLK - 1),
                                 skip_group_check=True)

    def rope_q(h, q_ps, c0, c1):
        # rope with bias folded in: out = ((q + bq) * trig)
        a = tmp.tile([P, BLK], F32, name="a", tag="a")
        nc.vector.scalar_tensor_tensor(
            a[:], q_ps[:], bqc[:, h:h + 1], c2[:, c0:c1], ADD, MULT)
        bb = tmp.tile([P, BLK], F32, name="bb", tag="bb")
        nc.vector.scalar_tensor_tensor(
            bb[0:64, :], q_ps[64:128, :], bqc[64:128, h:h + 1],
            s2x[64:128, c0:c1], ADD, MULT)
        nc.vector.scalar_tensor_tensor(
            bb[64:128, :], q_ps[0:64, :], bqc[0:64, h:h + 1],
            s2x[0:64, c0:c1], ADD, MULT)
        nc.vector.tensor_add(qtr[h][:, c0:c1], a[:], bb[:])

    def rope_k(k_ps, trig):
        cn = trig[:, 0:DLOC]
        sn = trig[:, DLOC:2 * DLOC]
        # rope(k + bk): bias added first (rope is linear), then rotate
        kb = tmp.tile([P, DLOC], F32, name="kb", tag="cc")
        nc.vector.tensor_add(kb[:], k_ps[:], bias["bkc"][:])
        a2 = tmp.tile([P, DLOC], F32, name="a2", tag="a")
        nc.vector.tensor_mul(a2[:], kb[:], cn[:])
        b2 = tmp.tile([P, DLOC], F32, name="b2", tag="bb")
        nc.vector.tensor_mul(pair_view(b2)[:, :, 0, :],
                             pair_view(kb)[:, :, 1, :],
                             pair_view(sn)[:, :, 1, :])
        nc.vector.tensor_mul(pair_view(b2)[:, :, 1, :],
                             pair_view(kb)[:, :, 0, :],
                             pair_view(sn)[:, :, 0, :])
        kr = kvpool.tile([P, DLOC], BF16, name="kr", tag="kr")
        nc.vector.tensor_add(kr[:], a2[:], b2[:])
        return kr

    mh_pend = None
    # --- phase B: projections + rope + Mh accumulation ---
    for blk in range(NBLK):
        c0, c1 = blk * BLK, (blk + 1) * BLK
        first = blk == 0 and xts is not None
        xt = xts[blk] if (xts is not None and blk in xts) else load_xt(blk)

        def xv(k):
            return xt[:, k * BLK:(k + 1) * BLK]

        def xl(k, m):
            return xt[:, k * BLK + m * 128:k * BLK + (m + 1) * 128]

        if first:
            # chunk-major block 0: K (4 concurrent accumulation banks)
            # consumes 0.5MB chunks as they arrive, then Q h0-2 (the other
            # 3 banks), then Q h3 / V as chains against now-resident
            # weights, landing on banks freed by the K/Q ropes in order;
            # ropes and Mh deferred to the tail of the block
            k_pss = [psum.tile([P, DLOC], F32, name="k_ps", tag="ps")
                     for _ in range(4)]
            for j in range(NCH):
                for m in range(4):
                    for k in range(j * KCH, (j + 1) * KCH):
                        nc.tensor.matmul(k_pss[m][:], xl(k, m),
                                         wk_all[:, k * DLOC:(k + 1) * DLOC],
                                         start=(k == 0), stop=(k == KT - 1),
                                         skip_group_check=True)
            q_pss = [psum.tile([P, BLK], F32, name="q_ps", tag="ps")
                     for _ in range(3)]
            for j in range(NCH):
                for h in range(3):
                    h0, h1 = h * 128, (h + 1) * 128
                    for k in range(j * KCH, (j + 1) * KCH):
                        nc.tensor.matmul(
                            q_pss[h][:],
                            wq_all[:, k * DLOC + h0:k * DLOC + h1],
                            xv(k), start=(k == 0), stop=(k == KT - 1),
                            skip_group_check=True)
            q_pss.append(psum.tile([P, BLK], F32, name="q_ps", tag="ps"))
            for k in range(KT):
                nc.tensor.matmul(q_pss[3][:],
                                 wq_all[:, k * DLOC + 384:k * DLOC + 512],
                                 xv(k), start=(k == 0), stop=(k == KT - 1),
                                 skip_group_check=True)
            krs = [rope_k(k_pss[m], trig0[m]) for m in range(4)]
            for h in range(NHL):
                rope_q(h, q_pss[h], c0, c1)
            vts = []
            for m in range(4):
                v_ps = psum.tile([P, DLOC], F32, name="v_ps", tag="ps")
                for k in range(KT):
                    nc.tensor.matmul(v_ps[:], xl(k, m),
                                     wv_all[:, k * DLOC:(k + 1) * DLOC],
                                     start=(k == 0), stop=(k == KT - 1),
                                     skip_group_check=True)
                vt = kvpool.tile([P, DLOC], BF16, name="vt", tag="vt")
                nc.vector.tensor_add(vt[:], v_ps[:], bias["bv_rep"][:])
                vts.append(vt)
            for m in range(4):
                emit_mh(krs[m], vts[m], m)
                if dbg is not None:
                    nc.sync.dma_start(dbg["kr"][m * P:(m + 1) * P, :], krs[m][:])
                    nc.sync.dma_start(dbg["v"][m * P:(m + 1) * P, :], vts[m][:])
            continue

        # steady-state blocks: Q head-chains, then K/V per m-tile with the
        # Mh matmuls software-pipelined one m behind the rope DVE chain
        for h in range(NHL):
            h0, h1 = h * 128, (h + 1) * 128
            q_ps = psum.tile([P, BLK], F32, name="q_ps", tag="ps")
            for k in range(KT):
                nc.tensor.matmul(q_ps[:],
                                 wq_all[:, k * DLOC + h0:k * DLOC + h1],
                                 xv(k), start=(k == 0), stop=(k == KT - 1),
                                 skip_group_check=True)
            rope_q(h, q_ps, c0, c1)

        for m in range(4):
            mg = blk * 4 + m
            r0 = mg * 128
            k_ps = psum.tile([P, DLOC], F32, name="k_ps", tag="ps")
            for k in range(KT):
                nc.tensor.matmul(k_ps[:], xl(k, m),
                                 wk_all[:, k * DLOC:(k + 1) * DLOC],
                                 start=(k == 0), stop=(k == KT - 1),
                                 skip_group_check=True)
            trig = tpool.tile([P, 2 * DLOC], BF16, name="trig", tag="trig")
            nc.sync.dma_start(trig[:], d["trigP"][r0:r0 + 128, :])
            kr = rope_k(k_ps, trig)

            v_ps = psum.tile([P, DLOC], F32, name="v_ps", tag="ps")
            for k in range(KT):
                nc.tensor.matmul(v_ps[:], xl(k, m),
                                 wv_all[:, k * DLOC:(k + 1) * DLOC],
                                 start=(k == 0), stop=(k == KT - 1),
                                 skip_group_check=True)
            vt = kvpool.tile([P, DLOC], BF16, name="vt", tag="vt")
            nc.vector.tensor_add(vt[:], v_ps[:], bias["bv_rep"][:])
            if dbg is not None:
                nc.sync.dma_start(dbg["kr"][r0:r0 + 128, :], kr[:])
                nc.sync.dma_start(dbg["v"][r0:r0 + 128, :], vt[:])

            if mh_pend is not None:
                emit_mh(*mh_pend)
            mh_pend = (kr, vt, mg)

    emit_mh(*mh_pend)
    # PSUM -> SBUF for Mh, interleaved per head across both copy engines so
    # each head's N matmuls start as soon as its 128-col copy lands
    m_sb = kvpool.tile([P, DLOC], BF16, name="m_sb", tag="m_sb", bufs=1)
    for h in range(NHL):
        h0, h1 = h * 128, (h + 1) * 128
        if h % 2 == 0:
            nc.scalar.copy(m_sb[:, h0:h1], mh_ps[:, h0:h1])
        else:
            nc.vector.tensor_copy(m_sb[:, h0:h1], mh_ps[:, h0:h1])
    if dbg is not None:
        nc.sync.dma_start(dbg["m"][:], m_sb[:])
        for h in range(NHL):
            nc.sync.dma_start(dbg["qtr"][h * 128:(h + 1) * 128, :], qtr[h][:])

    # --- phase D: N_h = Mh @ woT_h (tiny), then out = sum_h Q_h @ N_h ---
    n_sb = []
    for h in range(NHL):
        t = kvpool.tile([P, DIN], BF16, name=f"n_sb{h}", tag=f"n_sb{h}",
                        bufs=1)
        n_sb.append(t)
    for h in range(NHL):
        h0, h1 = h * 128, (h + 1) * 128
        for n in range(4):
            n_ps = psum.tile([P, 512], F32, name="n_ps", tag="ps")
            nc.tensor.matmul(n_ps[:], m_sb[:, h0:h1],
                             wo_all[:, h * DIN + n * 512:h * DIN + (n + 1) * 512],
                             start=True, stop=True, skip_group_check=True)
            if (h + n) % 2 == 0:
                nc.scalar.copy(n_sb[h][:, n * 512:(n + 1) * 512], n_ps[:])
            else:
                nc.vector.tensor_copy(n_sb[h][:, n * 512:(n + 1) * 512], n_ps[:])
    for blk in range(NBLK):
        c0 = blk * BLK
        for m in range(4):
            last = (blk == NBLK - 1 and m == 3)
            orow = osb.tile([P, DIN], BF16, name="orow", tag="ot")
            for n in range(4):
                o_ps = psum.tile([P, 512], F32, name="o_ps", tag="ps")
                for h in range(NHL):
                    nc.tensor.matmul(
                        o_ps[:], qtr[h][:, c0 + m * 128:c0 + (m + 1) * 128],
                        n_sb[h][:, n * 512:(n + 1) * 512],
                        start=(h == 0), stop=(h == NHL - 1),
                        skip_group_check=True)
                if (m + n) % 2 == 0:
                    nc.scalar.copy(orow[:, n * 512:(n + 1) * 512], o_ps[:])
                else:
                    nc.vector.tensor_copy(orow[:, n * 512:(n + 1) * 512], o_ps[:])
                if last:
                    # per-n-tile writes at the very end on the (idle) SP
                    # queue: the final DMA is 512 cols, not 2048,
                    # shortening the post-matmul tail
                    nc.sync.dma_start(
                        out_d[c0 + m * 128: c0 + (m + 1) * 128,
                              n * 512:(n + 1) * 512],
                        orow[:, n * 512:(n + 1) * 512])
            if not last:
                nc.scalar.dma_start(
                    out_d[c0 + m * 128: c0 + (m + 1) * 128, :], orow[:])


def build_nc(debug_taps=False, reps=1, loop_n=0):
    global _NC_CACHE
    if _NC_CACHE is not None and not debug_taps and reps == 1 and not loop_n:
        return _NC_CACHE
    nc = bacc.Bacc("TRN2", target_bir_lowering=False, debug=False)
    d = {
        "xP": nc.dram_tensor("xP", [NBLK * P, KT * BLK], BF16, kind="ExternalInput").ap(),
        "wqP": nc.dram_tensor("wqP", [P, KT * DLOC], BF16, kind="ExternalInput").ap(),
        "wkP": nc.dram_tensor("wkP", [P, KT * DLOC], BF16, kind="ExternalInput").ap(),
        "wvP": nc.dram_tensor("wvP", [P, KT * DLOC], BF16, kind="ExternalInput").ap(),
        "woP": nc.dram_tensor("woP", [P, NHL * DIN], BF16, kind="ExternalInput").ap(),
        "c2": nc.dram_tensor("c2", [P, S], BF16, kind="ExternalInput").ap(),
        "s2x": nc.dram_tensor("s2x", [P, S], BF16, kind="ExternalInput").ap(),
        "trigP": nc.dram_tensor("trigP", [S, 2 * DLOC], BF16, kind="ExternalInput").ap(),
        "bqc": nc.dram_tensor("bqc", [P, NHL], BF16, kind="ExternalInput").ap(),
        "bkc": nc.dram_tensor("bkc", [P, DLOC], BF16, kind="ExternalInput").ap(),
        "bv_rep": nc.dram_tensor("bv_rep", [P, DLOC], BF16, kind="ExternalInput").ap(),
    }
    out_d = nc.dram_tensor("out", [S, DIN], BF16, kind="ExternalOutput").ap()
    dbg = None
    if debug_taps:
        dbg = {
            "qtr": nc.dram_tensor("dbg_qtr", [DLOC, S], BF16, kind="ExternalOutput").ap(),
            "kr": nc.dram_tensor("dbg_kr", [S, DLOC], BF16, kind="ExternalOutput").ap(),
            "v": nc.dram_tensor("dbg_v", [S, DLOC], BF16, kind="ExternalOutput").ap(),
            "m": nc.dram_tensor("dbg_m", [P, DLOC], BF16, kind="ExternalOutput").ap(),
        }
    with tile.TileContext(nc) as tc, ExitStack() as ctx:
        _emit(nc, tc, ctx, d, out_d, dbg=dbg, reps=reps, loop_n=loop_n)
    nc.compile()
    if not debug_taps and reps == 1 and not loop_n:
        _NC_CACHE = nc
    return nc


def _pair_perm():
    # within each head: 64 even pair-elements then 64 odd
    idx = np.arange(DLOC).reshape(NHL, HD)
    return np.concatenate([idx[:, 0::2], idx[:, 1::2]], axis=1).reshape(-1)


def prep_in_maps(x, freqs_cos, freqs_sin, wq_w, wq_b, wk_w, wk_b,
                 wv_w, wv_b, wo_w, wo_b):
    x = np.asarray(x, np.float32)
    cos = np.asarray(freqs_cos, np.float32)
    sin = np.asarray(freqs_sin, np.float32)
    wq_w = np.asarray(wq_w, np.float32)
    wq_b = np.asarray(wq_b, np.float32)
    wk_w = np.asarray(wk_w, np.float32)
    wk_b = np.asarray(wk_b, np.float32)
    wv_w = np.asarray(wv_w, np.float32)
    wv_b = np.asarray(wv_b, np.float32)
    wo_w = np.asarray(wo_w, np.float32)

    cosT = np.ascontiguousarray(cos.T)          # [64, S]
    sinT = np.ascontiguousarray(sin.T)
    c2 = np.concatenate([cosT, cosT], axis=0).astype(NPBF)       # [128, S]
    s2x = np.concatenate([sinT, -sinT], axis=0).astype(NPBF)
    cn4 = np.tile(np.concatenate([cos, cos], axis=1), (1, NHL))
    sn4x = np.tile(np.concatenate([sin, -sin], axis=1), (1, NHL))
    trigP = np.ascontiguousarray(
        np.concatenate([cn4, sn4x], axis=1)).astype(NPBF)        # [S, 1024]

    def pack_w(wT):
        # [2048, F] -> [128, 16*F] with k-major column packing
        f = wT.shape[1]
        return np.ascontiguousarray(
            wT.reshape(KT, P, f).transpose(1, 0, 2).reshape(P, KT * f))

    perm = _pair_perm()
    sc = np.float32(1.0 / np.sqrt(HD))
    in_maps = []
    for c in range(NCORES):
        b, g = divmod(c, TP)
        sl = slice(g * DLOC, (g + 1) * DLOC)
        wq_p = (wq_w[sl][perm] * sc)
        bq_p = (wq_b[sl][perm] * sc)
        wk_p = wk_w[sl][perm]
        bk_p = wk_b[sl][perm]
        wv_p = wv_w[sl]
        bv_p = wv_b[sl]
        xT = np.ascontiguousarray(x[b].T)       # [2048, 2048]
        xP = np.ascontiguousarray(
            xT.reshape(KT, P, NBLK, BLK).transpose(2, 1, 0, 3)
              .reshape(NBLK * P, KT * BLK))
        in_maps.append({
            "xP": xP.astype(NPBF),
            "wqP": pack_w(wq_p.T).astype(NPBF),
            "wkP": pack_w(wk_p.T).astype(NPBF),
            "wvP": pack_w(wv_p.T).astype(NPBF),
            "woP": np.ascontiguousarray(
                wo_w[:, sl].T.reshape(NHL, P, DIN).transpose(1, 0, 2)
                .reshape(P, NHL * DIN)).astype(NPBF),
            "c2": c2, "s2x": s2x, "trigP": trigP,
            "bqc": np.ascontiguousarray(bq_p.reshape(NHL, P).T).astype(NPBF),
            "bkc": np.broadcast_to(bk_p[None, :], (P, DLOC)).astype(NPBF),
            "bv_rep": np.broadcast_to(bv_p[None, :], (P, DLOC)).astype(NPBF),
        })
    return in_maps


def assemble(results, wo_b):
    wo_b = np.asarray(wo_b, np.float32)
    out = np.zeros((B, S, DIN), np.float32)
    for c, r in enumerate(results):
        out[c // TP] += np.asarray(r["out"], np.float32)
    out += wo_b[None, None, :]
    return out


def kernel(**inputs):
    nc = build_nc()
    in_maps = prep_in_maps(
        inputs["x"], inputs["freqs_cos"], inputs["freqs_sin"],
        inputs["wq_w"], inputs["wq_b"], inputs["wk_w"], inputs["wk_b"],
        inputs["wv_w"], inputs["wv_b"], inputs["wo_w"], inputs["wo_b"])
    res = run_bass_kernel_spmd(nc, in_maps, core_ids=list(range(NCORES)))
    return assemble(res.results, inputs["wo_b"])
